# revision 22
# baseline (speedup 1.0000x reference)
"""Trainium2 Bass kernel for a 16-filter binarized 5x5 VALID conv.

Problem: x [B=32, C=6, H=512, W=512] f32; three grouped convs with
binarized 5x5 weights (channel subsets per output filter), concatenated
to out [32, 16, 508, 508] f32.

Mapping (per NeuronCore, data-parallel over batch, B/8 images each):
  conv == matmul with a banded block-Toeplitz stationary operand.
  x is staged in SBUF as 20-row windows (6 ch x 20 rows = 120
  partitions; window stride 16 rows). Each window yields 16 output rows
  via TWO band-shifted weight sets (separate PSUM banks):
    h=0: output rows w..w+7   (band over window rows 0..11)
    h=1: output rows w+8..w+15 (band over window rows 8..19)
  Per band group: M = 16 filters x 8 rows = 128 PSUM partitions
  (m = o*8 + r), N = W-4 = 508 columns (one PSUM bank f32); five
  matmuls (dx = kernel column; rhs = column-shifted slice of the window
  tile) accumulate into the bank, then one DVE tensor_scalar evacuation
  + one 8-row store.

  K=120 partitions per matmul matters: partial-K matmuls (e.g. 72)
  never un-throttle the PE HAM clock gate (measured 439 ns/MM vs
  232 ns/MM at K>=96 for N=508 bf16 on this part).

  Weights are binarized to sign(c)*alpha per filter. We ship exact +/-1
  sign matrices in bf16 (exact) and fold alpha into the evacuation as a
  per-partition tensor_scalar multiply, so the only quantization error
  is x's bf16 rounding (~1e-3 relative).

Windows overlap at image tails; overlapping rows are rewritten with
bitwise-identical values (same nonzero terms accumulated in the same
order — zero terms from the other band's matmuls add exactly 0.0).
"""

import numpy as np
import ml_dtypes

import concourse.bass as bass
import concourse.mybir as mybir
from concourse import bacc
from concourse import tile
from concourse.bass_utils import run_bass_kernel_spmd

MAPS3 = np.array([[0, 1, 2], [1, 2, 3], [2, 3, 4], [3, 4, 5], [0, 4, 5], [0, 1, 5]])
MAPS4 = np.array(
    [
        [0, 1, 2, 3],
        [1, 2, 3, 4],
        [2, 3, 4, 5],
        [0, 3, 4, 5],
        [0, 1, 4, 5],
        [0, 1, 2, 5],
        [0, 1, 3, 4],
        [1, 2, 4, 5],
        [0, 2, 3, 5],
    ]
)

C_IN = 6
N_OUT = 16
KH = KW = 5
R = 8  # output rows per band group
RH_WIN = 20  # input rows per SBUF window (2 groups, 4-row conv halo)
W_STRIDE = 16  # row stride between windows
KDIM = C_IN * RH_WIN  # 120 contraction partitions
MP = 128  # PSUM partitions per matmul: m = o*8 + r
N_H = 2  # band positions per window
N_CORES = 8
WCHUNK = 8  # windows per input-DMA chunk


def _binarize_np(w):
    """Mirror reference.binarize in numpy fp32: sign matrix + per-filter alpha."""
    w = np.asarray(w, dtype=np.float32)
    m = w - w.mean(axis=1, keepdims=True)
    c = np.clip(m, -1.0, 1.0)
    alpha = np.abs(c).mean(axis=(1, 2, 3))
    return np.sign(c).astype(np.float32), alpha.astype(np.float32)


def _filter_table(w3, w4, w6):
    """Per output filter: (channel list, sign[ci,dy,dx] fp32, alpha)."""
    s3, a3 = _binarize_np(w3)
    s4, a4 = _binarize_np(w4)
    s6, a6 = _binarize_np(w6)
    table = []
    for o in range(6):
        table.append((list(MAPS3[o]), s3[o], a3[o]))
    for o in range(9):
        table.append((list(MAPS4[o]), s4[o], a4[o]))
    table.append((list(range(6)), s6[0], a6[0]))
    return table


def _build_weight_inputs(w3, w4, w6):
    """wm [KDIM, N_H*KW*128] bf16 sign matrices (slice (h*KW+dx) is the
    [120,128] stationary operand), alphas [128,1] f32 per (o,r)."""
    table = _filter_table(w3, w4, w6)
    wm = np.zeros((KDIM, N_H, KW, MP), dtype=np.float32)
    alphas = np.zeros((MP, 1), dtype=np.float32)
    for o, (chans, sgn, alpha) in enumerate(table):
        for r in range(R):
            m = o * R + r
            alphas[m, 0] = alpha
            for h in range(N_H):
                for ci, c in enumerate(chans):
                    for dy in range(KH):
                        k = c * RH_WIN + R * h + r + dy
                        wm[k, h, :, m] = sgn[ci, dy, :]
    wm = wm.reshape(KDIM, N_H * KW * MP).astype(ml_dtypes.bfloat16)
    return wm, alphas


def _group_starts(h_out):
    assert h_out >= R
    starts = []
    s = 0
    while True:
        starts.append(min(s, h_out - R))
        if s >= h_out - R:
            break
        s += R
    return starts


def _window_plan(h, h_out):
    """Window starts covering all output rows; each window emits 16 rows
    (w..w+15). Returns (main_ws, extra_ws): main_ws is a stride-16 run
    (one batched DMA per channel per chunk), extra_ws are stragglers."""
    per = []
    for s in _group_starts(h_out):
        if s % W_STRIDE == 0 and s + RH_WIN <= h:
            w = s
        else:
            w = s - R
            assert w >= 0 and w + RH_WIN <= h, (s, h)
        if w not in per:
            per.append(w)
    ws = sorted(per)
    # every window writes rows [w, w+16) -> must fit in the output
    assert all(w + 2 * R <= h_out for w in ws)
    main_ws = []
    for i, w in enumerate(ws):
        if w == i * W_STRIDE:
            main_ws.append(w)
        else:
            break
    extra_ws = ws[len(main_ws) :]
    return main_ws, extra_ws


def build_nc(b_per_core, h, w, num_cores=N_CORES):
    """Build + compile the per-core Bass program."""
    h_out, w_out = h - KH + 1, w - KW + 1
    assert w_out <= 512
    f32 = mybir.dt.float32
    bf16 = mybir.dt.bfloat16

    main_ws, extra_ws = _window_plan(h, h_out)
    # split the main run into chunks of WCHUNK windows
    chunks = [main_ws[i : i + WCHUNK] for i in range(0, len(main_ws), WCHUNK)]

    nc = bacc.Bacc(
        "TRN2",
        target_bir_lowering=False,
        debug=False,
        num_devices=num_cores,
    )
    x_t = nc.dram_tensor("xb", [b_per_core, C_IN, h, w], bf16, kind="ExternalInput")
    wm_t = nc.dram_tensor("wm", [KDIM, N_H * KW * MP], bf16, kind="ExternalInput")
    al_t = nc.dram_tensor("alphas", [MP, 1], f32, kind="ExternalInput")
    out_t = nc.dram_tensor(
        "out", [b_per_core, N_OUT, h_out, w_out], f32, kind="ExternalOutput"
    )

    with tile.TileContext(nc) as tc:
        with (
            tc.tile_pool(name="wpool", bufs=1) as wpool,
            tc.tile_pool(name="xpool", bufs=16) as xpool,
            tc.tile_pool(name="tpool", bufs=8) as tpool,
            tc.tile_pool(name="spool", bufs=8) as spool,
            tc.tile_pool(name="ppool", bufs=8, space="PSUM") as ppool,
        ):
            wt = wpool.tile([KDIM, N_H * KW * MP], bf16, tag="wt")
            nc.sync.dma_start(out=wt[:], in_=wm_t[:])
            at = wpool.tile([MP, 1], f32, tag="at")
            nc.sync.dma_start(out=at[:], in_=al_t[:])

            def do_window(xt, wl, b, wstart):
                """Two band groups: 5 accumulating matmuls + alpha-scale +
                8-row store each."""
                for hb in range(N_H):
                    ps = ppool.tile([MP, w_out], f32, tag="ps")
                    for dx in range(KW):
                        sl = (hb * KW + dx) * MP
                        nc.tensor.matmul(
                            ps[:],
                            wt[:, sl : sl + MP],
                            xt[:, wl * w + dx : wl * w + dx + w_out],
                            start=(dx == 0),
                            stop=(dx == KW - 1),
                        )
                    st = spool.tile([MP, w_out], f32, tag="st")
                    nc.vector.tensor_scalar_mul(st[:], ps[:], at[:])
                    dst = bass.AP(
                        out_t,
                        b * N_OUT * h_out * w_out + (wstart + R * hb) * w_out,
                        [[h_out * w_out, N_OUT], [w_out, R], [1, w_out]],
                    )
                    nc.scalar.dma_start(out=dst, in_=st[:])

            def load_chunk(b, ci):
                """Emit the 6 per-channel DMAs for chunk ci of batch b."""
                chunk = chunks[ci]
                nwc = len(chunk)
                xt = xpool.tile([KDIM, nwc * w], bf16, tag="xt", name=f"xt_{b}_{ci}")
                for c in range(C_IN):
                    src = bass.AP(
                        x_t,
                        b * C_IN * h * w + c * h * w + chunk[0] * w,
                        [[w, RH_WIN], [W_STRIDE * w, nwc], [1, w]],
                    )
                    eng = nc.sync if c % 2 == 0 else nc.scalar
                    eng.dma_start(
                        out=xt[c * RH_WIN : (c + 1) * RH_WIN, :], in_=src
                    )
                return xt

            def load_tail(b, wstart):
                xt2 = tpool.tile(
                    [KDIM, w], bf16, tag="xt_tail", name=f"xtt_{b}_{wstart}"
                )
                src = bass.AP(
                    x_t,
                    b * C_IN * h * w + wstart * w,
                    [[h * w, C_IN], [w, RH_WIN], [1, w]],
                )
                nc.scalar.dma_start(out=xt2[:], in_=src)
                return xt2

            # preload the first two batches; pace the rest between windows
            LOOKAHEAD = 2
            xtiles = {}
            ttiles = {}
            for b in range(min(LOOKAHEAD, b_per_core)):
                for ci in range(len(chunks)):
                    xtiles[(b, ci)] = load_chunk(b, ci)
                for wstart in extra_ws:
                    ttiles[(b, wstart)] = load_tail(b, wstart)

            for b in range(b_per_core):
                bl = b + LOOKAHEAD
                for ci, chunk in enumerate(chunks):
                    if bl < b_per_core:
                        xtiles[(bl, ci)] = load_chunk(bl, ci)
                    xt = xtiles.pop((b, ci))
                    for wl, wstart in enumerate(chunk):
                        do_window(xt, wl, b, wstart)
                for wstart in extra_ws:
                    if bl < b_per_core:
                        ttiles[(bl, wstart)] = load_tail(bl, wstart)
                    do_window(ttiles.pop((b, wstart)), 0, b, wstart)

    nc.compile()
    return nc


_NC_CACHE = {}


def _get_nc(b_per_core, h, w):
    key = (b_per_core, h, w)
    if key not in _NC_CACHE:
        _NC_CACHE[key] = build_nc(b_per_core, h, w)
    return _NC_CACHE[key]


def _prep_inputs(x, w3, w4, w6):
    b = x.shape[0]
    assert b % N_CORES == 0
    bpc = b // N_CORES
    wm, alphas = _build_weight_inputs(w3, w4, w6)
    xb = np.ascontiguousarray(x).astype(ml_dtypes.bfloat16)
    in_maps = [
        {
            "xb": np.ascontiguousarray(xb[i * bpc : (i + 1) * bpc]),
            "wm": wm,
            "alphas": alphas,
        }
        for i in range(N_CORES)
    ]
    return bpc, in_maps


def run(x, w3, w4, w6, trace=False, **kw):
    b, c, h, w = x.shape
    bpc, in_maps = _prep_inputs(x, w3, w4, w6)
    nc = _get_nc(bpc, h, w)
    res = run_bass_kernel_spmd(
        nc, in_maps, list(range(N_CORES)), trace=trace, **kw
    )
    out = np.concatenate([r["out"] for r in res.results], axis=0)
    return np.asarray(out, dtype=np.float32), res


def kernel(x, w3, w4, w6):
    out, _ = run(x, w3, w4, w6, trace=False)
    return out
